# revision 2
# baseline (speedup 1.0000x reference)
"""MLA (DeepSeek-V2-Lite, absorbed) forward kernel for 8 Trainium2 NeuronCores.

v2: transpose-free dataflow. All projections run weight-stationary over a
resident xT [DIM, tokens] so every output lands directly in [feature, token]
layout (qT_nope, qT_pe, k_peT, kv_latT). Attention scores are computed
TRANSPOSED (scT[t, s] = kv_latT^T... via stationary kv_latT blocks, moving
q_latT), so exp() writes attn^T directly and the 544 PE attn-transposes of
the v1 kernel disappear. Softmax denominators come from a ones-column
matmul over attn^T; the 1/sum scale is folded into the xl PSUM evacuation
multiply. The only PE transposes left are the 128 kv_latT -> kv_lat[t,c]
block transposes (both layouts are genuinely needed) plus tiny rstd rows.

Sharding: tensor-parallel over heads (2 heads/core), partial wo outputs
summed on the host.
"""

import sys

for _p in ("/opt/trn_rl_repo",):
    if _p not in sys.path:
        sys.path.append(_p)

import numpy as np
import ml_dtypes

import concourse.bacc as bacc
import concourse.tile as tile
import concourse.mybir as mybir
from concourse import bass_utils

BF16 = mybir.dt.bfloat16
F32 = mybir.dt.float32
AF = mybir.ActivationFunctionType

DIM = 2048
H = 16
C = 512          # kv_lora_rank
NOPE = 128
R = 64           # rope dim
V = 128          # v_head_dim
QK = NOPE + R
B = 2
S = 2048
N_CORES = 8
HL = H // N_CORES   # heads per core (2)
P = 128
DT = DIM // P       # 16 K-tiles over model dim
CT = C // P         # 4 c-blocks
NBLK = 8            # projection M-blocks: kv0-3, kpe, ropes, qn0, qn1
CH = 512            # projection token-chunk (one PSUM bank)
NEG = -1.0e30


def _emit_rope_t(nc, pool, ps, pbase, cs4, cs4sw, out_ev, out_od, tag):
    """Rope on transposed layout. ps rows [pbase:pbase+32] = even coeffs,
    [pbase+32:pbase+64] = odd (PSUM f32). cs4 rows 0:32 = cos, 32:64 = sin;
    cs4sw rows 0:32 = sin, 32:64 = cos. Walrus requires equal base
    partitions when BOTH tensor-tensor inputs are SBUF, so the partition
    shift always rides the PSUM operand (exempt) and every SBUF operand
    of an op shares its base."""
    RH = R // 2
    w1 = pool.tile([R, CH], F32, tag=tag + "w1", bufs=1)
    w2 = pool.tile([R, CH], F32, tag=tag + "w2", bufs=1)
    ev = ps[pbase:pbase + RH, :]
    od = ps[pbase + RH:pbase + 2 * RH, :]
    nc.vector.tensor_mul(w1[0:RH, :], ev, cs4[0:RH, :])        # ev*cos @b0
    nc.vector.tensor_mul(w2[0:RH, :], od, cs4sw[0:RH, :])      # od*sin @b0
    nc.vector.tensor_mul(w1[RH:R, :], ev, cs4[RH:R, :])        # ev*sin @b32
    nc.vector.tensor_mul(w2[RH:R, :], od, cs4sw[RH:R, :])      # od*cos @b32
    nc.vector.tensor_sub(out_ev, w1[0:RH, :], w2[0:RH, :])
    nc.vector.tensor_add(out_od, w1[RH:R, :], w2[RH:R, :])


def build_nc(s_per_b=S, n_cores=N_CORES):
    ST = s_per_b // P          # s-tiles per batch (16)
    TT = B * ST                # total token tiles (32)
    NTOK = B * s_per_b         # 4096
    NCH = NTOK // CH           # 8 chunks

    nc = bacc.Bacc("TRN2", target_bir_lowering=False, debug=False,
                   num_devices=n_cores)

    xT_d = nc.dram_tensor("xT", [DIM, NTOK], BF16, kind="ExternalInput").ap()
    wall_d = nc.dram_tensor("wall", [DIM, NBLK, P], BF16,
                            kind="ExternalInput").ap()
    wkvb1_d = nc.dram_tensor("wkvb1", [NOPE, HL, C], BF16,
                             kind="ExternalInput").ap()
    wkvb2_d = nc.dram_tensor("wkvb2", [C, HL, V], BF16,
                             kind="ExternalInput").ap()
    woT_d = nc.dram_tensor("woT", [V, HL, DIM], BF16,
                           kind="ExternalInput").ap()
    cs4_d = nc.dram_tensor("cs4", [P, s_per_b], F32, kind="ExternalInput").ap()
    cs4sw_d = nc.dram_tensor("cs4sw", [P, s_per_b], F32,
                             kind="ExternalInput").ap()
    ident_d = nc.dram_tensor("ident", [P, P], BF16, kind="ExternalInput").ap()
    maskT_d = nc.dram_tensor("maskT", [P, HL, P], BF16,
                             kind="ExternalInput").ap()
    y_d = nc.dram_tensor("y", [NTOK, DIM], F32, kind="ExternalOutput").ap()

    with tile.TileContext(nc) as tc:
        with tc.tile_pool(name="static", bufs=1) as st:
            ident_sb = st.tile([P, P], BF16)
            nc.sync.dma_start(out=ident_sb, in_=ident_d)
            maskT_sb = st.tile([P, HL, P], BF16)
            nc.sync.dma_start(out=maskT_sb, in_=maskT_d)
            ones_col = st.tile([P, 1], BF16)
            nc.vector.memset(ones_col, 1.0)
            eps_sb = st.tile([P, 1], F32)
            nc.vector.memset(eps_sb, 1e-6)

            # residents
            kv_latT_sb = st.tile([P, CT, NTOK], BF16)    # [c%128, kc, t]
            kv_lat_sb = st.tile([P, TT, C], BF16)        # [t%128, ti, c]
            k_peT_sb = st.tile([R, NTOK], BF16)          # [r', t]
            qT_pe_sb = st.tile([R, HL, NTOK], BF16)      # [r', h, t]
            qT_nope_sb = st.tile([P, HL, NTOK], BF16)    # [d, h, t]
            wkvb1_sb = st.tile([NOPE, HL, C], BF16)
            nc.sync.dma_start(out=wkvb1_sb, in_=wkvb1_d)
            wkvb2_sb = st.tile([P, CT, HL, V], BF16)
            nc.sync.dma_start(
                out=wkvb2_sb, in_=wkvb2_d.rearrange("(kc p) h v -> p kc h v",
                                                    p=P))
            woT_sb = st.tile([V, HL, DIM], BF16)
            nc.sync.dma_start(out=woT_sb, in_=woT_d)

            # ================= PHASE 1: projections =================
            with tc.tile_pool(name="p1", bufs=1) as p1, \
                 tc.tile_pool(name="p1ps", bufs=1, space="PSUM") as p1ps:
                wall_sb = p1.tile([P, DT, NBLK, P], BF16)
                nc.sync.dma_start(
                    out=wall_sb,
                    in_=wall_d.rearrange("(kd p) b m -> p kd b m", p=P))

                xT_r = xT_d.rearrange("(kd p) t -> p kd t", p=P)
                for ch in range(NCH):
                    c0 = ch * CH
                    pos0 = (ch % (NCH // B)) * CH  # position within batch
                    xch = p1.tile([P, DT, CH], BF16, tag="xch", bufs=2)
                    nc.sync.dma_start(out=xch, in_=xT_r[:, :, c0:c0 + CH])
                    cs4 = p1.tile([P, CH], F32, tag="cs4", bufs=2)
                    nc.sync.dma_start(out=cs4, in_=cs4_d[:, pos0:pos0 + CH])
                    cs4sw = p1.tile([P, CH], F32, tag="cs4sw", bufs=2)
                    nc.sync.dma_start(out=cs4sw,
                                      in_=cs4sw_d[:, pos0:pos0 + CH])

                    for blk in range(NBLK):
                        ps = p1ps.tile([P, CH], F32, tag="proj", bufs=3)
                        for kd in range(DT):
                            nc.tensor.matmul(ps, wall_sb[:, kd, blk, :],
                                             xch[:, kd, :],
                                             start=(kd == 0),
                                             stop=(kd == DT - 1))
                        if blk < CT:
                            # kv latent block -> raw bf16 (normalized later)
                            nc.vector.tensor_copy(
                                out=kv_latT_sb[:, blk, c0:c0 + CH], in_=ps)
                        elif blk == CT:
                            # k_pe rows 0:64 = [ev|od]; rope it
                            _emit_rope_t(nc, p1, ps, 0, cs4, cs4sw,
                                         k_peT_sb[0:R // 2, c0:c0 + CH],
                                         k_peT_sb[R // 2:R, c0:c0 + CH],
                                         "kpe")
                        elif blk == CT + 1:
                            # q ropes: rows [h0(ev|od) | h1(ev|od)]
                            for h in range(HL):
                                _emit_rope_t(
                                    nc, p1, ps, h * R, cs4, cs4sw,
                                    qT_pe_sb[0:R // 2, h, c0:c0 + CH],
                                    qT_pe_sb[R // 2:R, h, c0:c0 + CH],
                                    f"qpe{h}")
                        else:
                            h = blk - (CT + 2)
                            nc.vector.tensor_copy(
                                out=qT_nope_sb[:, h, c0:c0 + CH], in_=ps)

                    # kv: transpose to [t, c], RMSNorm there, then scale the
                    # [c, t] copy by rstd via transpose+broadcast.
                    for tj in range(CH // P):
                        ti = (c0 + tj * P) // P
                        tcol = c0 + tj * P
                        for kc in range(CT):
                            tp = p1ps.tile([P, P], BF16, tag="tp", bufs=2)
                            nc.tensor.transpose(
                                tp, kv_latT_sb[:, kc, tcol:tcol + P], ident_sb)
                            nc.vector.tensor_copy(
                                out=kv_lat_sb[:, ti, kc * P:(kc + 1) * P],
                                in_=tp)
                        sq = p1.tile([P, C], BF16, tag="sq", bufs=2)
                        ssum = p1.tile([P, 1], F32, tag="ssum", bufs=2)
                        nc.scalar.activation(out=sq, in_=kv_lat_sb[:, ti, :],
                                             func=AF.Square, accum_out=ssum)
                        rstd = p1.tile([P, 1], F32, tag="rstd", bufs=2)
                        nc.scalar.activation(out=rstd, in_=ssum, func=AF.Sqrt,
                                             bias=eps_sb, scale=1.0 / C)
                        nc.vector.reciprocal(rstd, rstd)
                        nc.vector.tensor_scalar_mul(
                            out=kv_lat_sb[:, ti, :], in0=kv_lat_sb[:, ti, :],
                            scalar1=rstd)
                        # rstd as a row -> broadcast -> scale kv_latT in place
                        rstd_bf = p1.tile([P, 1], BF16, tag="rstdbf", bufs=2)
                        nc.vector.tensor_copy(out=rstd_bf, in_=rstd)
                        rT_ps = p1ps.tile([1, P], BF16, tag="rT", bufs=2)
                        nc.tensor.transpose(rT_ps, rstd_bf, ident_sb)
                        rT_sb = p1.tile([1, P], BF16, tag="rTsb", bufs=2)
                        nc.vector.tensor_copy(out=rT_sb, in_=rT_ps)
                        rbc = p1.tile([P, P], BF16, tag="rbc", bufs=2)
                        nc.gpsimd.partition_broadcast(rbc, rT_sb)
                        for kc in range(CT):
                            nc.vector.tensor_mul(
                                kv_latT_sb[:, kc, tcol:tcol + P],
                                kv_latT_sb[:, kc, tcol:tcol + P], rbc)

            # ================= PHASE 2: attention + output =================
            with tc.tile_pool(name="p2", bufs=1) as p2, \
                 tc.tile_pool(name="p2ps", bufs=1, space="PSUM") as p2ps:
                for b in range(B):
                    for i in range(ST):
                        gi = b * ST + i
                        scol = gi * P
                        nj = i + 1
                        # q_latT for this row-tile (both heads, on the fly)
                        qlat_sb = p2.tile([P, CT, HL, P], BF16, tag="qlatsb",
                                          bufs=2)
                        for kcp in range(2):
                            qlat_ps = p2ps.tile([P, 2, HL, P], F32, tag="sc",
                                                bufs=2)
                            for kk in range(2):
                                kc = kcp * 2 + kk
                                for h in range(HL):
                                    nc.tensor.matmul(
                                        qlat_ps[:, kk, h, :],
                                        wkvb1_sb[:, h, kc * P:(kc + 1) * P],
                                        qT_nope_sb[:, h, scol:scol + P],
                                        start=True, stop=True)
                            nc.vector.tensor_copy(
                                out=qlat_sb[:, kcp * 2:kcp * 2 + 2, :, :],
                                in_=qlat_ps)

                        attnT = p2.tile([P, ST, HL, P], BF16, tag="attnT",
                                        bufs=2)
                        sume_ps = p2ps.tile([1, HL, P], F32, tag="sume",
                                            bufs=1)
                        xl_ps = p2ps.tile([P, CT, HL, P], F32, tag="xl",
                                          bufs=1)
                        for j0 in range(0, nj, 2):
                            w = min(2, nj - j0)
                            scps = p2ps.tile([P, 2, HL, P], F32, tag="sc",
                                             bufs=2)
                            for jj in range(w):
                                j = j0 + jj
                                tcol = (b * ST + j) * P
                                for kc in range(CT):
                                    nc.tensor.matmul(
                                        scps[:, jj, :, :],
                                        kv_latT_sb[:, kc, tcol:tcol + P],
                                        qlat_sb[:, kc, :, :],
                                        start=(kc == 0), stop=False)
                                nc.tensor.matmul(
                                    scps[:, jj, :, :],
                                    k_peT_sb[:, tcol:tcol + P],
                                    qT_pe_sb[:, :, scol:scol + P],
                                    start=False, stop=True)
                                if j == i:
                                    nc.vector.tensor_add(
                                        scps[:, jj, :, :], scps[:, jj, :, :],
                                        maskT_sb)
                            nc.scalar.activation(
                                out=attnT[:, j0:j0 + w, :, :],
                                in_=scps[:, 0:w, :, :], func=AF.Exp)
                            for jj in range(w):
                                j = j0 + jj
                                nc.tensor.matmul(
                                    sume_ps, ones_col, attnT[:, j, :, :],
                                    start=(j == 0), stop=(j == nj - 1))

                        for kc in range(CT):
                            for j in range(nj):
                                nc.tensor.matmul(
                                    xl_ps[:, kc, :, :],
                                    kv_lat_sb[:, b * ST + j,
                                              kc * P:(kc + 1) * P],
                                    attnT[:, j, :, :],
                                    start=(j == 0), stop=(j == nj - 1))

                        recip = p2.tile([1, HL, P], F32, tag="recip", bufs=2)
                        nc.vector.reciprocal(recip, sume_ps)
                        rbc2 = p2.tile([P, HL, P], F32, tag="rbc2", bufs=2)
                        nc.gpsimd.partition_broadcast(rbc2, recip)
                        xl_sb = p2.tile([P, CT, HL, P], BF16, tag="xlsb",
                                        bufs=2)
                        for kc in range(CT):
                            nc.vector.tensor_mul(
                                xl_sb[:, kc, :, :], xl_ps[:, kc, :, :], rbc2)

                        outT_ps = p2ps.tile([V, HL, P], F32, tag="outT",
                                            bufs=1)
                        for h in range(HL):
                            for kc in range(CT):
                                nc.tensor.matmul(
                                    outT_ps[:, h, :],
                                    wkvb2_sb[:, kc, h, :],
                                    xl_sb[:, kc, h, :],
                                    start=(kc == 0), stop=(kc == CT - 1))
                        outT_sb = p2.tile([V, HL, P], BF16, tag="outTsb",
                                          bufs=2)
                        nc.vector.tensor_copy(out=outT_sb, in_=outT_ps)

                        y_sb = p2.tile([P, DIM], F32, tag="ysb", bufs=2)
                        for m0 in range(0, DIM, 512):
                            y_ps = p2ps.tile([P, 512], F32, tag="yps", bufs=2)
                            for h in range(HL):
                                nc.tensor.matmul(
                                    y_ps, outT_sb[:, h, :],
                                    woT_sb[:, h, m0:m0 + 512],
                                    start=(h == 0), stop=(h == HL - 1))
                            nc.scalar.copy(out=y_sb[:, m0:m0 + 512], in_=y_ps)
                        nc.sync.dma_start(
                            out=y_d[scol:scol + P, :], in_=y_sb)

    nc.compile()
    return nc


def _deinterleave(w):
    """[64, DIM] interleaved rope rows -> [ev(32) | od(32)]."""
    return np.concatenate([w[0::2], w[1::2]], axis=0)


def shard_inputs(x, freqs_cis, wq, wkv_a, wkv_b, wo, kv_norm_w,
                 s_per_b=S, n_cores=N_CORES):
    bf16 = ml_dtypes.bfloat16
    scale = np.float32(QK ** -0.5)

    xf = np.asarray(x, np.float32).reshape(B * s_per_b, DIM)
    xT = np.ascontiguousarray(xf.T.astype(bf16))           # [DIM, NTOK]

    fc = np.asarray(freqs_cis, np.float32)
    cosT = np.ascontiguousarray(fc[:, :, 0].T)             # [32, S]
    sinT = np.ascontiguousarray(fc[:, :, 1].T)
    cs4 = np.concatenate([cosT, sinT, cosT, sinT], axis=0)     # [128, S]
    cs4sw = np.concatenate([sinT, cosT, sinT, cosT], axis=0)

    wqf = np.asarray(wq, np.float32)                       # [H*QK, DIM]
    wkva = np.asarray(wkv_a, np.float32)                   # [C+R, DIM]
    wkvb = np.asarray(wkv_b, np.float32).reshape(H, NOPE + V, C)
    wof = np.asarray(wo, np.float32)                       # [DIM, H*V]
    wn = np.asarray(kv_norm_w, np.float32)                 # [C]

    kpe_blk = np.concatenate(
        [_deinterleave(wkva[C:C + R]), np.zeros((R, DIM), np.float32)], axis=0)

    ident = np.eye(P, dtype=bf16)
    ii = np.arange(P)
    maskT = np.where(ii[:, None] <= ii[None, :], 0.0, NEG).astype(np.float32)
    maskT2 = np.ascontiguousarray(
        np.broadcast_to(maskT[:, None, :], (P, HL, P))).astype(bf16)

    in_maps = []
    for c in range(n_cores):
        h0 = c * HL
        wq_c = wqf.reshape(H, QK, DIM)[h0:h0 + HL] * scale  # [HL, QK, DIM]
        ropes = np.concatenate(
            [_deinterleave(wq_c[h, NOPE:]) for h in range(HL)], axis=0)
        blocks = [wkva[kc * P:(kc + 1) * P] for kc in range(CT)]
        blocks += [kpe_blk, ropes, wq_c[0, :NOPE], wq_c[1, :NOPE]]
        wall = np.stack(blocks, axis=0)                    # [NBLK, 128, DIM]
        wall = np.ascontiguousarray(
            wall.transpose(2, 0, 1)).astype(bf16)          # [DIM, NBLK, 128]

        b1 = (wkvb[h0:h0 + HL, :NOPE, :] * wn[None, None, :])  # [HL,128,C]
        wkvb1 = np.ascontiguousarray(b1.transpose(1, 0, 2)).astype(bf16)
        b2 = (wkvb[h0:h0 + HL, NOPE:, :] * wn[None, None, :])  # [HL,V,C]
        wkvb2 = np.ascontiguousarray(b2.transpose(2, 0, 1)).astype(bf16)
        woT_c = np.ascontiguousarray(
            wof[:, h0 * V:(h0 + HL) * V].T.reshape(HL, V, DIM)
            .transpose(1, 0, 2)).astype(bf16)              # [V, HL, DIM]
        in_maps.append({
            "xT": xT,
            "wall": wall,
            "wkvb1": wkvb1,
            "wkvb2": wkvb2,
            "woT": woT_c,
            "cs4": cs4,
            "cs4sw": cs4sw,
            "ident": ident,
            "maskT": maskT2,
        })
    return in_maps


_NC_CACHE = {}


def get_nc(s_per_b=S):
    if s_per_b not in _NC_CACHE:
        _NC_CACHE[s_per_b] = build_nc(s_per_b)
    return _NC_CACHE[s_per_b]


def kernel(x, freqs_cis, wq, wkv_a, wkv_b, wo, kv_norm_w, trace=False):
    nc = get_nc(S)
    in_maps = shard_inputs(x, freqs_cis, wq, wkv_a, wkv_b, wo, kv_norm_w)
    res = bass_utils.run_bass_kernel_spmd(
        nc, in_maps, core_ids=list(range(N_CORES)), trace=trace)
    y = res.results[0]["y"].astype(np.float64)
    for i in range(1, N_CORES):
        y += res.results[i]["y"]
    out = y.astype(np.float32).reshape(B, S, DIM)
    if trace:
        kernel.last_exec_time_ns = res.exec_time_ns
        kernel.last_results = res
    return out


# revision 3
# speedup vs baseline: 1.1566x; 1.1566x over previous
"""MLA (DeepSeek-V2-Lite, absorbed) forward kernel for 8 Trainium2 NeuronCores.

v3: transpose-free dataflow, row-tile-paired attention (every attention matmul streams N=512 so the serial LDWEIGHTS issue cost hides under the moving stream). All projections run weight-stationary over a
resident xT [DIM, tokens] so every output lands directly in [feature, token]
layout (qT_nope, qT_pe, k_peT, kv_latT). Attention scores are computed
TRANSPOSED (scT[t, s] = kv_latT^T... via stationary kv_latT blocks, moving
q_latT), so exp() writes attn^T directly and the 544 PE attn-transposes of
the v1 kernel disappear. Softmax denominators come from a ones-column
matmul over attn^T; the 1/sum scale is folded into the xl PSUM evacuation
multiply. The only PE transposes left are the 128 kv_latT -> kv_lat[t,c]
block transposes (both layouts are genuinely needed) plus tiny rstd rows.

Sharding: tensor-parallel over heads (2 heads/core), partial wo outputs
summed on the host.
"""

import sys

for _p in ("/opt/trn_rl_repo",):
    if _p not in sys.path:
        sys.path.append(_p)

import numpy as np
import ml_dtypes

import concourse.bacc as bacc
import concourse.tile as tile
import concourse.mybir as mybir
from concourse import bass_utils

BF16 = mybir.dt.bfloat16
F32 = mybir.dt.float32
AF = mybir.ActivationFunctionType

DIM = 2048
H = 16
C = 512          # kv_lora_rank
NOPE = 128
R = 64           # rope dim
V = 128          # v_head_dim
QK = NOPE + R
B = 2
S = 2048
N_CORES = 8
HL = H // N_CORES   # heads per core (2)
P = 128
DT = DIM // P       # 16 K-tiles over model dim
CT = C // P         # 4 c-blocks
NBLK = 8            # projection M-blocks: kv0-3, kpe, ropes, qn0, qn1
CH = 512            # projection token-chunk (one PSUM bank)
NEG = -1.0e30


def _emit_rope_t(nc, pool, ps, pbase, cs4, cs4sw, out_ev, out_od, tag):
    """Rope on transposed layout. ps rows [pbase:pbase+32] = even coeffs,
    [pbase+32:pbase+64] = odd (PSUM f32). cs4 rows 0:32 = cos, 32:64 = sin;
    cs4sw rows 0:32 = sin, 32:64 = cos. Walrus requires equal base
    partitions when BOTH tensor-tensor inputs are SBUF, so the partition
    shift always rides the PSUM operand (exempt) and every SBUF operand
    of an op shares its base."""
    RH = R // 2
    w1 = pool.tile([R, CH], F32, tag=tag + "w1", bufs=1)
    w2 = pool.tile([R, CH], F32, tag=tag + "w2", bufs=1)
    ev = ps[pbase:pbase + RH, :]
    od = ps[pbase + RH:pbase + 2 * RH, :]
    nc.vector.tensor_mul(w1[0:RH, :], ev, cs4[0:RH, :])        # ev*cos @b0
    nc.vector.tensor_mul(w2[0:RH, :], od, cs4sw[0:RH, :])      # od*sin @b0
    nc.vector.tensor_mul(w1[RH:R, :], ev, cs4[RH:R, :])        # ev*sin @b32
    nc.vector.tensor_mul(w2[RH:R, :], od, cs4sw[RH:R, :])      # od*cos @b32
    nc.vector.tensor_sub(out_ev, w1[0:RH, :], w2[0:RH, :])
    nc.vector.tensor_add(out_od, w1[RH:R, :], w2[RH:R, :])


def build_nc(s_per_b=S, n_cores=N_CORES):
    ST = s_per_b // P          # s-tiles per batch (16)
    TT = B * ST                # total token tiles (32)
    NTOK = B * s_per_b         # 4096
    NCH = NTOK // CH           # 8 chunks

    nc = bacc.Bacc("TRN2", target_bir_lowering=False, debug=False,
                   num_devices=n_cores)

    xT_d = nc.dram_tensor("xT", [DIM, NTOK], BF16, kind="ExternalInput").ap()
    wall_d = nc.dram_tensor("wall", [DIM, NBLK, P], BF16,
                            kind="ExternalInput").ap()
    wkvb1_d = nc.dram_tensor("wkvb1", [NOPE, HL, C], BF16,
                             kind="ExternalInput").ap()
    wkvb2_d = nc.dram_tensor("wkvb2", [C, HL, V], BF16,
                             kind="ExternalInput").ap()
    woT_d = nc.dram_tensor("woT", [V, HL, DIM], BF16,
                           kind="ExternalInput").ap()
    cs4_d = nc.dram_tensor("cs4", [P, s_per_b], F32, kind="ExternalInput").ap()
    cs4sw_d = nc.dram_tensor("cs4sw", [P, s_per_b], F32,
                             kind="ExternalInput").ap()
    ident_d = nc.dram_tensor("ident", [P, P], BF16, kind="ExternalInput").ap()
    maskA_d = nc.dram_tensor("maskA", [P, HL, 2, P], BF16,
                             kind="ExternalInput").ap()
    maskB_d = nc.dram_tensor("maskB", [P, HL, 2, P], BF16,
                             kind="ExternalInput").ap()
    y_d = nc.dram_tensor("y", [NTOK, DIM], F32, kind="ExternalOutput").ap()

    with tile.TileContext(nc) as tc:
        with tc.tile_pool(name="static", bufs=1) as st:
            ident_sb = st.tile([P, P], BF16)
            nc.sync.dma_start(out=ident_sb, in_=ident_d)
            maskA_sb = st.tile([P, HL, 2, P], BF16)
            nc.sync.dma_start(out=maskA_sb, in_=maskA_d)
            maskB_sb = st.tile([P, HL, 2, P], BF16)
            nc.sync.dma_start(out=maskB_sb, in_=maskB_d)
            ones_col = st.tile([P, 1], BF16)
            nc.vector.memset(ones_col, 1.0)
            eps_sb = st.tile([P, 1], F32)
            nc.vector.memset(eps_sb, 1e-6)

            # residents
            kv_latT_sb = st.tile([P, CT, NTOK], BF16)    # [c%128, kc, t]
            kv_lat_sb = st.tile([P, TT, C], BF16)        # [t%128, ti, c]
            k_peT_sb = st.tile([R, NTOK], BF16)          # [r', t]
            qT_pe_sb = st.tile([R, HL, NTOK], BF16)      # [r', h, t]
            qT_nope_sb = st.tile([P, HL, NTOK], BF16)    # [d, h, t]
            wkvb1_sb = st.tile([NOPE, HL, C], BF16)
            nc.sync.dma_start(out=wkvb1_sb, in_=wkvb1_d)
            wkvb2_sb = st.tile([P, CT, HL, V], BF16)
            nc.sync.dma_start(
                out=wkvb2_sb, in_=wkvb2_d.rearrange("(kc p) h v -> p kc h v",
                                                    p=P))
            woT_sb = st.tile([V, HL, DIM], BF16)
            nc.sync.dma_start(out=woT_sb, in_=woT_d)

            # ================= PHASE 1: projections =================
            with tc.tile_pool(name="p1", bufs=1) as p1, \
                 tc.tile_pool(name="p1ps", bufs=1, space="PSUM") as p1ps:
                wall_sb = p1.tile([P, DT, NBLK, P], BF16)
                nc.sync.dma_start(
                    out=wall_sb,
                    in_=wall_d.rearrange("(kd p) b m -> p kd b m", p=P))

                xT_r = xT_d.rearrange("(kd p) t -> p kd t", p=P)
                for ch in range(NCH):
                    c0 = ch * CH
                    pos0 = (ch % (NCH // B)) * CH  # position within batch
                    xch = p1.tile([P, DT, CH], BF16, tag="xch", bufs=2)
                    nc.sync.dma_start(out=xch, in_=xT_r[:, :, c0:c0 + CH])
                    cs4 = p1.tile([P, CH], F32, tag="cs4", bufs=2)
                    nc.sync.dma_start(out=cs4, in_=cs4_d[:, pos0:pos0 + CH])
                    cs4sw = p1.tile([P, CH], F32, tag="cs4sw", bufs=2)
                    nc.sync.dma_start(out=cs4sw,
                                      in_=cs4sw_d[:, pos0:pos0 + CH])

                    for blk in range(NBLK):
                        ps = p1ps.tile([P, CH], F32, tag="proj", bufs=3)
                        for kd in range(DT):
                            nc.tensor.matmul(ps, wall_sb[:, kd, blk, :],
                                             xch[:, kd, :],
                                             start=(kd == 0),
                                             stop=(kd == DT - 1))
                        if blk < CT:
                            # kv latent block -> raw bf16 (normalized later)
                            nc.vector.tensor_copy(
                                out=kv_latT_sb[:, blk, c0:c0 + CH], in_=ps)
                        elif blk == CT:
                            # k_pe rows 0:64 = [ev|od]; rope it
                            _emit_rope_t(nc, p1, ps, 0, cs4, cs4sw,
                                         k_peT_sb[0:R // 2, c0:c0 + CH],
                                         k_peT_sb[R // 2:R, c0:c0 + CH],
                                         "kpe")
                        elif blk == CT + 1:
                            # q ropes: rows [h0(ev|od) | h1(ev|od)]
                            for h in range(HL):
                                _emit_rope_t(
                                    nc, p1, ps, h * R, cs4, cs4sw,
                                    qT_pe_sb[0:R // 2, h, c0:c0 + CH],
                                    qT_pe_sb[R // 2:R, h, c0:c0 + CH],
                                    f"qpe{h}")
                        else:
                            h = blk - (CT + 2)
                            nc.vector.tensor_copy(
                                out=qT_nope_sb[:, h, c0:c0 + CH], in_=ps)

                    # kv: transpose to [t, c], RMSNorm there, then scale the
                    # [c, t] copy by rstd via transpose+broadcast.
                    for tj in range(CH // P):
                        ti = (c0 + tj * P) // P
                        tcol = c0 + tj * P
                        for kc in range(CT):
                            tp = p1ps.tile([P, P], BF16, tag="tp", bufs=2)
                            nc.tensor.transpose(
                                tp, kv_latT_sb[:, kc, tcol:tcol + P], ident_sb)
                            nc.vector.tensor_copy(
                                out=kv_lat_sb[:, ti, kc * P:(kc + 1) * P],
                                in_=tp)
                        sq = p1.tile([P, C], BF16, tag="sq", bufs=2)
                        ssum = p1.tile([P, 1], F32, tag="ssum", bufs=2)
                        nc.scalar.activation(out=sq, in_=kv_lat_sb[:, ti, :],
                                             func=AF.Square, accum_out=ssum)
                        rstd = p1.tile([P, 1], F32, tag="rstd", bufs=2)
                        nc.scalar.activation(out=rstd, in_=ssum, func=AF.Sqrt,
                                             bias=eps_sb, scale=1.0 / C)
                        nc.vector.reciprocal(rstd, rstd)
                        nc.vector.tensor_scalar_mul(
                            out=kv_lat_sb[:, ti, :], in0=kv_lat_sb[:, ti, :],
                            scalar1=rstd)
                        # rstd as a row -> broadcast -> scale kv_latT in place
                        rstd_bf = p1.tile([P, 1], BF16, tag="rstdbf", bufs=2)
                        nc.vector.tensor_copy(out=rstd_bf, in_=rstd)
                        rT_ps = p1ps.tile([1, P], BF16, tag="rT", bufs=2)
                        nc.tensor.transpose(rT_ps, rstd_bf, ident_sb)
                        rT_sb = p1.tile([1, P], BF16, tag="rTsb", bufs=2)
                        nc.vector.tensor_copy(out=rT_sb, in_=rT_ps)
                        rbc = p1.tile([P, P], BF16, tag="rbc", bufs=2)
                        nc.gpsimd.partition_broadcast(rbc, rT_sb)
                        for kc in range(CT):
                            nc.vector.tensor_mul(
                                kv_latT_sb[:, kc, tcol:tcol + P],
                                kv_latT_sb[:, kc, tcol:tcol + P], rbc)

            # ================= PHASE 2: attention + output =================
            with tc.tile_pool(name="p2", bufs=1) as p2, \
                 tc.tile_pool(name="p2ps", bufs=1, space="PSUM") as p2ps:
                for b in range(B):
                    for pr in range(ST // 2):
                        r0 = 2 * pr
                        r1 = r0 + 1
                        gi0 = b * ST + r0
                        scol = gi0 * P          # 256 token columns
                        nj = r1 + 1
                        # q_latT for both row-tiles, both heads
                        qlat_sb = p2.tile([P, CT, HL, 2, P], BF16,
                                          tag="qlatsb", bufs=2)
                        for kc in range(CT):
                            qlat_ps = p2ps.tile([P, HL, 2, P], F32, tag="sc",
                                                bufs=2)
                            for h in range(HL):
                                nc.tensor.matmul(
                                    qlat_ps[:, h, :, :],
                                    wkvb1_sb[:, h, kc * P:(kc + 1) * P],
                                    qT_nope_sb[:, h, scol:scol + 2 * P],
                                    start=True, stop=True)
                            nc.vector.tensor_copy(
                                out=qlat_sb[:, kc], in_=qlat_ps)

                        attnT = p2.tile([P, ST, HL, 2, P], BF16, tag="attnT",
                                        bufs=2)
                        sume_ps = p2ps.tile([1, HL, 2, P], F32, tag="sume",
                                            bufs=1)
                        for j in range(nj):
                            tcol = (b * ST + j) * P
                            scps = p2ps.tile([P, HL, 2, P], F32, tag="sc",
                                             bufs=2)
                            for kc in range(CT):
                                nc.tensor.matmul(
                                    scps,
                                    kv_latT_sb[:, kc, tcol:tcol + P],
                                    qlat_sb[:, kc],
                                    start=(kc == 0), stop=False)
                            nc.tensor.matmul(
                                scps,
                                k_peT_sb[:, tcol:tcol + P],
                                qT_pe_sb[:, :, scol:scol + 2 * P],
                                start=False, stop=True)
                            if j == r0:
                                nc.vector.tensor_add(scps, scps, maskA_sb)
                            elif j == r1:
                                nc.vector.tensor_add(scps, scps, maskB_sb)
                            nc.scalar.activation(
                                out=attnT[:, j], in_=scps, func=AF.Exp)
                            nc.tensor.matmul(
                                sume_ps, ones_col, attnT[:, j],
                                start=(j == 0), stop=(j == nj - 1))

                        recip = p2.tile([1, HL, 2, P], F32, tag="recip",
                                        bufs=2)
                        nc.vector.reciprocal(recip, sume_ps)
                        rbc2 = p2.tile([P, HL, 2, P], F32, tag="rbc2", bufs=2)
                        nc.gpsimd.partition_broadcast(rbc2, recip)
                        xl_sb = p2.tile([P, CT, HL, 2, P], BF16, tag="xlsb",
                                        bufs=2)
                        for kc in range(CT):
                            xl_ps = p2ps.tile([P, HL, 2, P], F32, tag="xl",
                                              bufs=2)
                            for j in range(nj):
                                nc.tensor.matmul(
                                    xl_ps,
                                    kv_lat_sb[:, b * ST + j,
                                              kc * P:(kc + 1) * P],
                                    attnT[:, j],
                                    start=(j == 0), stop=(j == nj - 1))
                            nc.vector.tensor_mul(xl_sb[:, kc], xl_ps, rbc2)

                        outT_ps = p2ps.tile([V, HL, 2, P], F32, tag="outT",
                                            bufs=1)
                        for h in range(HL):
                            for kc in range(CT):
                                nc.tensor.matmul(
                                    outT_ps[:, h],
                                    wkvb2_sb[:, kc, h, :],
                                    xl_sb[:, kc, h],
                                    start=(kc == 0), stop=(kc == CT - 1))
                        outT_sb = p2.tile([V, HL, 2, P], BF16, tag="outTsb",
                                          bufs=2)
                        nc.vector.tensor_copy(out=outT_sb, in_=outT_ps)

                        for tt in range(2):
                            gi = gi0 + tt
                            y_sb = p2.tile([P, DIM], F32, tag="ysb", bufs=2)
                            for m0 in range(0, DIM, 512):
                                y_ps = p2ps.tile([P, 512], F32, tag="yps",
                                                 bufs=2)
                                for h in range(HL):
                                    nc.tensor.matmul(
                                        y_ps, outT_sb[:, h, tt, :],
                                        woT_sb[:, h, m0:m0 + 512],
                                        start=(h == 0), stop=(h == HL - 1))
                                nc.scalar.copy(out=y_sb[:, m0:m0 + 512],
                                               in_=y_ps)
                            nc.sync.dma_start(
                                out=y_d[gi * P:(gi + 1) * P, :], in_=y_sb)

    nc.compile()
    return nc


def _deinterleave(w):
    """[64, DIM] interleaved rope rows -> [ev(32) | od(32)]."""
    return np.concatenate([w[0::2], w[1::2]], axis=0)


def shard_inputs(x, freqs_cis, wq, wkv_a, wkv_b, wo, kv_norm_w,
                 s_per_b=S, n_cores=N_CORES):
    bf16 = ml_dtypes.bfloat16
    scale = np.float32(QK ** -0.5)

    xf = np.asarray(x, np.float32).reshape(B * s_per_b, DIM)
    xT = np.ascontiguousarray(xf.T.astype(bf16))           # [DIM, NTOK]

    fc = np.asarray(freqs_cis, np.float32)
    cosT = np.ascontiguousarray(fc[:, :, 0].T)             # [32, S]
    sinT = np.ascontiguousarray(fc[:, :, 1].T)
    cs4 = np.concatenate([cosT, sinT, cosT, sinT], axis=0)     # [128, S]
    cs4sw = np.concatenate([sinT, cosT, sinT, cosT], axis=0)

    wqf = np.asarray(wq, np.float32)                       # [H*QK, DIM]
    wkva = np.asarray(wkv_a, np.float32)                   # [C+R, DIM]
    wkvb = np.asarray(wkv_b, np.float32).reshape(H, NOPE + V, C)
    wof = np.asarray(wo, np.float32)                       # [DIM, H*V]
    wn = np.asarray(kv_norm_w, np.float32)                 # [C]

    kpe_blk = np.concatenate(
        [_deinterleave(wkva[C:C + R]), np.zeros((R, DIM), np.float32)], axis=0)

    ident = np.eye(P, dtype=bf16)
    ii = np.arange(P)
    tri = np.where(ii[:, None] <= ii[None, :], 0.0, NEG).astype(np.float32)
    maskA = np.zeros((P, HL, 2, P), np.float32)
    maskA[:, :, 0, :] = tri[:, None, :]
    maskB = np.full((P, HL, 2, P), NEG, np.float32)
    maskB[:, :, 1, :] = tri[:, None, :]
    maskA = maskA.astype(bf16)
    maskB = maskB.astype(bf16)

    in_maps = []
    for c in range(n_cores):
        h0 = c * HL
        wq_c = wqf.reshape(H, QK, DIM)[h0:h0 + HL] * scale  # [HL, QK, DIM]
        ropes = np.concatenate(
            [_deinterleave(wq_c[h, NOPE:]) for h in range(HL)], axis=0)
        blocks = [wkva[kc * P:(kc + 1) * P] for kc in range(CT)]
        blocks += [kpe_blk, ropes, wq_c[0, :NOPE], wq_c[1, :NOPE]]
        wall = np.stack(blocks, axis=0)                    # [NBLK, 128, DIM]
        wall = np.ascontiguousarray(
            wall.transpose(2, 0, 1)).astype(bf16)          # [DIM, NBLK, 128]

        b1 = (wkvb[h0:h0 + HL, :NOPE, :] * wn[None, None, :])  # [HL,128,C]
        wkvb1 = np.ascontiguousarray(b1.transpose(1, 0, 2)).astype(bf16)
        b2 = (wkvb[h0:h0 + HL, NOPE:, :] * wn[None, None, :])  # [HL,V,C]
        wkvb2 = np.ascontiguousarray(b2.transpose(2, 0, 1)).astype(bf16)
        woT_c = np.ascontiguousarray(
            wof[:, h0 * V:(h0 + HL) * V].T.reshape(HL, V, DIM)
            .transpose(1, 0, 2)).astype(bf16)              # [V, HL, DIM]
        in_maps.append({
            "xT": xT,
            "wall": wall,
            "wkvb1": wkvb1,
            "wkvb2": wkvb2,
            "woT": woT_c,
            "cs4": cs4,
            "cs4sw": cs4sw,
            "ident": ident,
            "maskA": maskA,
            "maskB": maskB,
        })
    return in_maps


_NC_CACHE = {}


def get_nc(s_per_b=S):
    if s_per_b not in _NC_CACHE:
        _NC_CACHE[s_per_b] = build_nc(s_per_b)
    return _NC_CACHE[s_per_b]


def kernel(x, freqs_cis, wq, wkv_a, wkv_b, wo, kv_norm_w, trace=False):
    nc = get_nc(S)
    in_maps = shard_inputs(x, freqs_cis, wq, wkv_a, wkv_b, wo, kv_norm_w)
    res = bass_utils.run_bass_kernel_spmd(
        nc, in_maps, core_ids=list(range(N_CORES)), trace=trace)
    y = res.results[0]["y"].astype(np.float64)
    for i in range(1, N_CORES):
        y += res.results[i]["y"]
    out = y.astype(np.float32).reshape(B, S, DIM)
    if trace:
        kernel.last_exec_time_ns = res.exec_time_ns
        kernel.last_results = res
    return out


# revision 4
# speedup vs baseline: 1.1700x; 1.0116x over previous
"""MLA (DeepSeek-V2-Lite, absorbed) forward kernel for 8 Trainium2 NeuronCores.

v3: transpose-free dataflow, row-tile-paired attention (every attention matmul streams N=512 so the serial LDWEIGHTS issue cost hides under the moving stream). All projections run weight-stationary over a
resident xT [DIM, tokens] so every output lands directly in [feature, token]
layout (qT_nope, qT_pe, k_peT, kv_latT). Attention scores are computed
TRANSPOSED (scT[t, s] = kv_latT^T... via stationary kv_latT blocks, moving
q_latT), so exp() writes attn^T directly and the 544 PE attn-transposes of
the v1 kernel disappear. Softmax denominators come from a ones-column
matmul over attn^T; the 1/sum scale is folded into the xl PSUM evacuation
multiply. The only PE transposes left are the 128 kv_latT -> kv_lat[t,c]
block transposes (both layouts are genuinely needed) plus tiny rstd rows.

Sharding: tensor-parallel over heads (2 heads/core), partial wo outputs
summed on the host.
"""

import sys

for _p in ("/opt/trn_rl_repo",):
    if _p not in sys.path:
        sys.path.append(_p)

import numpy as np
import ml_dtypes

import concourse.bacc as bacc
import concourse.tile as tile
import concourse.mybir as mybir
from concourse import bass_utils

BF16 = mybir.dt.bfloat16
F32 = mybir.dt.float32
AF = mybir.ActivationFunctionType

DIM = 2048
H = 16
C = 512          # kv_lora_rank
NOPE = 128
R = 64           # rope dim
V = 128          # v_head_dim
QK = NOPE + R
B = 2
S = 2048
N_CORES = 8
HL = H // N_CORES   # heads per core (2)
P = 128
DT = DIM // P       # 16 K-tiles over model dim
CT = C // P         # 4 c-blocks
NBLK = 8            # projection M-blocks: kv0-3, kpe, ropes, qn0, qn1
CH = 512            # projection token-chunk (one PSUM bank)
NEG = -1.0e30


def _emit_rope_t(nc, pool, ps, pbase, cs4, cs4sw, out_ev, out_od, tag):
    """Rope on transposed layout. ps rows [pbase:pbase+32] = even coeffs,
    [pbase+32:pbase+64] = odd (PSUM f32). cs4 rows 0:32 = cos, 32:64 = sin;
    cs4sw rows 0:32 = sin, 32:64 = cos. Walrus requires equal base
    partitions when BOTH tensor-tensor inputs are SBUF, so the partition
    shift always rides the PSUM operand (exempt) and every SBUF operand
    of an op shares its base."""
    RH = R // 2
    w1 = pool.tile([R, CH], F32, tag=tag + "w1", bufs=1)
    w2 = pool.tile([R, CH], F32, tag=tag + "w2", bufs=1)
    ev = ps[pbase:pbase + RH, :]
    od = ps[pbase + RH:pbase + 2 * RH, :]
    nc.vector.tensor_mul(w1[0:RH, :], ev, cs4[0:RH, :])        # ev*cos @b0
    nc.vector.tensor_mul(w2[0:RH, :], od, cs4sw[0:RH, :])      # od*sin @b0
    nc.vector.tensor_mul(w1[RH:R, :], ev, cs4[RH:R, :])        # ev*sin @b32
    nc.vector.tensor_mul(w2[RH:R, :], od, cs4sw[RH:R, :])      # od*cos @b32
    nc.vector.tensor_sub(out_ev, w1[0:RH, :], w2[0:RH, :])
    nc.vector.tensor_add(out_od, w1[RH:R, :], w2[RH:R, :])


def build_nc(s_per_b=S, n_cores=N_CORES):
    ST = s_per_b // P          # s-tiles per batch (16)
    TT = B * ST                # total token tiles (32)
    NTOK = B * s_per_b         # 4096
    NCH = NTOK // CH           # 8 chunks

    nc = bacc.Bacc("TRN2", target_bir_lowering=False, debug=False,
                   num_devices=n_cores)

    xT_d = nc.dram_tensor("xT", [DIM, NTOK], BF16, kind="ExternalInput").ap()
    wall_d = nc.dram_tensor("wall", [DIM, NBLK, P], BF16,
                            kind="ExternalInput").ap()
    wkvb1_d = nc.dram_tensor("wkvb1", [NOPE, HL, C], BF16,
                             kind="ExternalInput").ap()
    wkvb2_d = nc.dram_tensor("wkvb2", [C, HL, V], BF16,
                             kind="ExternalInput").ap()
    woT_d = nc.dram_tensor("woT", [V, HL, DIM], BF16,
                           kind="ExternalInput").ap()
    cs4_d = nc.dram_tensor("cs4", [P, s_per_b], F32, kind="ExternalInput").ap()
    cs4sw_d = nc.dram_tensor("cs4sw", [P, s_per_b], F32,
                             kind="ExternalInput").ap()
    ident_d = nc.dram_tensor("ident", [P, P], BF16, kind="ExternalInput").ap()
    maskA_d = nc.dram_tensor("maskA", [P, HL, 2, P], BF16,
                             kind="ExternalInput").ap()
    maskB_d = nc.dram_tensor("maskB", [P, HL, 2, P], BF16,
                             kind="ExternalInput").ap()
    y_d = nc.dram_tensor("y", [NTOK, DIM], F32, kind="ExternalOutput").ap()

    with tile.TileContext(nc) as tc:
        with tc.tile_pool(name="static", bufs=1) as st:
            ident_sb = st.tile([P, P], BF16)
            nc.sync.dma_start(out=ident_sb, in_=ident_d)
            maskA_sb = st.tile([P, HL, 2, P], BF16)
            nc.sync.dma_start(out=maskA_sb, in_=maskA_d)
            maskB_sb = st.tile([P, HL, 2, P], BF16)
            nc.sync.dma_start(out=maskB_sb, in_=maskB_d)
            ones_col = st.tile([P, 1], BF16)
            nc.vector.memset(ones_col, 1.0)
            eps_sb = st.tile([P, 1], F32)
            nc.vector.memset(eps_sb, 1e-6)

            # residents
            kv_latT_sb = st.tile([P, CT, NTOK], BF16)    # [c%128, kc, t]
            kv_lat_sb = st.tile([P, TT, C], BF16)        # [t%128, ti, c]
            k_peT_sb = st.tile([R, NTOK], BF16)          # [r', t]
            qT_pe_sb = st.tile([R, HL, NTOK], BF16)      # [r', h, t]
            qT_nope_sb = st.tile([P, HL, NTOK], BF16)    # [d, h, t]
            wkvb1_sb = st.tile([NOPE, HL, C], BF16)
            nc.sync.dma_start(out=wkvb1_sb, in_=wkvb1_d)
            wkvb2_sb = st.tile([P, CT, HL, V], BF16)
            nc.sync.dma_start(
                out=wkvb2_sb, in_=wkvb2_d.rearrange("(kc p) h v -> p kc h v",
                                                    p=P))
            woT_sb = st.tile([V, HL, DIM], BF16)
            nc.sync.dma_start(out=woT_sb, in_=woT_d)

            # ================= PHASE 1: projections =================
            with tc.tile_pool(name="p1", bufs=1) as p1, \
                 tc.tile_pool(name="p1ps", bufs=1, space="PSUM") as p1ps:
                wall_sb = p1.tile([P, DT, NBLK, P], BF16)
                nc.sync.dma_start(
                    out=wall_sb,
                    in_=wall_d.rearrange("(kd p) b m -> p kd b m", p=P))

                xT_r = xT_d.rearrange("(kd p) t -> p kd t", p=P)
                for ch in range(NCH):
                    c0 = ch * CH
                    pos0 = (ch % (NCH // B)) * CH  # position within batch
                    xch = p1.tile([P, DT, CH], BF16, tag="xch", bufs=2)
                    nc.sync.dma_start(out=xch, in_=xT_r[:, :, c0:c0 + CH])
                    cs4 = p1.tile([P, CH], F32, tag="cs4", bufs=2)
                    nc.sync.dma_start(out=cs4, in_=cs4_d[:, pos0:pos0 + CH])
                    cs4sw = p1.tile([P, CH], F32, tag="cs4sw", bufs=2)
                    nc.sync.dma_start(out=cs4sw,
                                      in_=cs4sw_d[:, pos0:pos0 + CH])

                    for blk in range(NBLK):
                        ps = p1ps.tile([P, CH], F32, tag="proj", bufs=3)
                        for kd in range(DT):
                            nc.tensor.matmul(ps, wall_sb[:, kd, blk, :],
                                             xch[:, kd, :],
                                             start=(kd == 0),
                                             stop=(kd == DT - 1))
                        if blk < CT:
                            # kv latent block -> raw bf16 (normalized later)
                            nc.vector.tensor_copy(
                                out=kv_latT_sb[:, blk, c0:c0 + CH], in_=ps)
                        elif blk == CT:
                            # k_pe rows 0:64 = [ev|od]; rope it
                            _emit_rope_t(nc, p1, ps, 0, cs4, cs4sw,
                                         k_peT_sb[0:R // 2, c0:c0 + CH],
                                         k_peT_sb[R // 2:R, c0:c0 + CH],
                                         "kpe")
                        elif blk == CT + 1:
                            # q ropes: rows [h0(ev|od) | h1(ev|od)]
                            for h in range(HL):
                                _emit_rope_t(
                                    nc, p1, ps, h * R, cs4, cs4sw,
                                    qT_pe_sb[0:R // 2, h, c0:c0 + CH],
                                    qT_pe_sb[R // 2:R, h, c0:c0 + CH],
                                    f"qpe{h}")
                        else:
                            h = blk - (CT + 2)
                            nc.vector.tensor_copy(
                                out=qT_nope_sb[:, h, c0:c0 + CH], in_=ps)

                    # kv: transpose to [t, c], RMSNorm there, then scale the
                    # [c, t] copy by rstd via transpose+broadcast.
                    for tj in range(CH // P):
                        ti = (c0 + tj * P) // P
                        tcol = c0 + tj * P
                        for kc in range(CT):
                            tp = p1ps.tile([P, P], BF16, tag="tp", bufs=2)
                            nc.tensor.transpose(
                                tp, kv_latT_sb[:, kc, tcol:tcol + P], ident_sb)
                            nc.vector.tensor_copy(
                                out=kv_lat_sb[:, ti, kc * P:(kc + 1) * P],
                                in_=tp)
                        sq = p1.tile([P, C], BF16, tag="sq", bufs=2)
                        ssum = p1.tile([P, 1], F32, tag="ssum", bufs=2)
                        nc.scalar.activation(out=sq, in_=kv_lat_sb[:, ti, :],
                                             func=AF.Square, accum_out=ssum)
                        rstd = p1.tile([P, 1], F32, tag="rstd", bufs=2)
                        nc.scalar.activation(out=rstd, in_=ssum, func=AF.Sqrt,
                                             bias=eps_sb, scale=1.0 / C)
                        nc.vector.reciprocal(rstd, rstd)
                        nc.vector.tensor_scalar_mul(
                            out=kv_lat_sb[:, ti, :], in0=kv_lat_sb[:, ti, :],
                            scalar1=rstd)
                        # rstd as a row -> broadcast -> scale kv_latT in place
                        rstd_bf = p1.tile([P, 1], BF16, tag="rstdbf", bufs=2)
                        nc.vector.tensor_copy(out=rstd_bf, in_=rstd)
                        rT_ps = p1ps.tile([1, P], BF16, tag="rT", bufs=2)
                        nc.tensor.transpose(rT_ps, rstd_bf, ident_sb)
                        rT_sb = p1.tile([1, P], BF16, tag="rTsb", bufs=2)
                        nc.vector.tensor_copy(out=rT_sb, in_=rT_ps)
                        rbc = p1.tile([P, P], BF16, tag="rbc", bufs=2)
                        nc.gpsimd.partition_broadcast(rbc, rT_sb)
                        for kc in range(CT):
                            nc.vector.tensor_mul(
                                kv_latT_sb[:, kc, tcol:tcol + P],
                                kv_latT_sb[:, kc, tcol:tcol + P], rbc)

            # ================= PHASE 2: attention + output =================
            with tc.tile_pool(name="p2", bufs=1) as p2, \
                 tc.tile_pool(name="p2ps", bufs=1, space="PSUM") as p2ps:
                def emit_qlat(b, pr):
                    # q_latT for both row-tiles of the pair, both heads
                    scol = (b * ST + 2 * pr) * P
                    qlat_sb = p2.tile([P, CT, HL, 2, P], BF16,
                                      tag="qlatsb", bufs=2, name="qlat_sb")
                    for kc in range(CT):
                        qlat_ps = p2ps.tile([P, HL, 2, P], F32, tag="sc",
                                            bufs=2, name="qlat_ps")
                        for h in range(HL):
                            nc.tensor.matmul(
                                qlat_ps[:, h, :, :],
                                wkvb1_sb[:, h, kc * P:(kc + 1) * P],
                                qT_nope_sb[:, h, scol:scol + 2 * P],
                                start=True, stop=True)
                        nc.vector.tensor_copy(
                            out=qlat_sb[:, kc], in_=qlat_ps)
                    return qlat_sb

                pairs = [(b, pr) for b in range(B) for pr in range(ST // 2)]
                qlat_sb = emit_qlat(*pairs[0])
                for idx, (b, pr) in enumerate(pairs):
                    if True:
                        r0 = 2 * pr
                        r1 = r0 + 1
                        gi0 = b * ST + r0
                        scol = gi0 * P          # 256 token columns
                        nj = r1 + 1

                        attnT = p2.tile([P, ST, HL, 2, P], BF16, tag="attnT",
                                        bufs=2)
                        sume_ps = p2ps.tile([1, HL, 2, P], F32, tag="sume",
                                            bufs=1)
                        for j in range(nj):
                            tcol = (b * ST + j) * P
                            scps = p2ps.tile([P, HL, 2, P], F32, tag="sc",
                                             bufs=2)
                            for kc in range(CT):
                                nc.tensor.matmul(
                                    scps,
                                    kv_latT_sb[:, kc, tcol:tcol + P],
                                    qlat_sb[:, kc],
                                    start=(kc == 0), stop=False)
                            nc.tensor.matmul(
                                scps,
                                k_peT_sb[:, tcol:tcol + P],
                                qT_pe_sb[:, :, scol:scol + 2 * P],
                                start=False, stop=True)
                            if j == r0:
                                nc.vector.tensor_add(scps, scps, maskA_sb)
                            elif j == r1:
                                nc.vector.tensor_add(scps, scps, maskB_sb)
                            nc.scalar.activation(
                                out=attnT[:, j], in_=scps, func=AF.Exp)
                            nc.tensor.matmul(
                                sume_ps, ones_col, attnT[:, j],
                                start=(j == 0), stop=(j == nj - 1))

                        recip = p2.tile([1, HL, 2, P], F32, tag="recip",
                                        bufs=2)
                        nc.vector.reciprocal(recip, sume_ps)
                        rbc2 = p2.tile([P, HL, 2, P], F32, tag="rbc2", bufs=2)
                        nc.gpsimd.partition_broadcast(rbc2, recip)
                        xl_sb = p2.tile([P, CT, HL, 2, P], BF16, tag="xlsb",
                                        bufs=2)
                        for kc in range(CT):
                            xl_ps = p2ps.tile([P, HL, 2, P], F32, tag="xl",
                                              bufs=2)
                            for j in range(nj):
                                nc.tensor.matmul(
                                    xl_ps,
                                    kv_lat_sb[:, b * ST + j,
                                              kc * P:(kc + 1) * P],
                                    attnT[:, j],
                                    start=(j == 0), stop=(j == nj - 1))
                            nc.vector.tensor_mul(xl_sb[:, kc], xl_ps, rbc2)

                        # prefetch next pair's q_lat into the PE gap
                        # while the recip/broadcast/evac chain completes
                        next_qlat = (emit_qlat(*pairs[idx + 1])
                                     if idx + 1 < len(pairs) else None)

                        outT_ps = p2ps.tile([V, HL, 2, P], F32, tag="outT",
                                            bufs=1)
                        for h in range(HL):
                            for kc in range(CT):
                                nc.tensor.matmul(
                                    outT_ps[:, h],
                                    wkvb2_sb[:, kc, h, :],
                                    xl_sb[:, kc, h],
                                    start=(kc == 0), stop=(kc == CT - 1))
                        outT_sb = p2.tile([V, HL, 2, P], BF16, tag="outTsb",
                                          bufs=2)
                        nc.vector.tensor_copy(out=outT_sb, in_=outT_ps)

                        for tt in range(2):
                            gi = gi0 + tt
                            y_sb = p2.tile([P, DIM], F32, tag="ysb", bufs=2)
                            for m0 in range(0, DIM, 512):
                                y_ps = p2ps.tile([P, 512], F32, tag="yps",
                                                 bufs=2)
                                for h in range(HL):
                                    nc.tensor.matmul(
                                        y_ps, outT_sb[:, h, tt, :],
                                        woT_sb[:, h, m0:m0 + 512],
                                        start=(h == 0), stop=(h == HL - 1))
                                nc.scalar.copy(out=y_sb[:, m0:m0 + 512],
                                               in_=y_ps)
                            nc.sync.dma_start(
                                out=y_d[gi * P:(gi + 1) * P, :], in_=y_sb)
                        if next_qlat is not None:
                            qlat_sb = next_qlat

    nc.compile()
    return nc


def _deinterleave(w):
    """[64, DIM] interleaved rope rows -> [ev(32) | od(32)]."""
    return np.concatenate([w[0::2], w[1::2]], axis=0)


def shard_inputs(x, freqs_cis, wq, wkv_a, wkv_b, wo, kv_norm_w,
                 s_per_b=S, n_cores=N_CORES):
    bf16 = ml_dtypes.bfloat16
    scale = np.float32(QK ** -0.5)

    xf = np.asarray(x, np.float32).reshape(B * s_per_b, DIM)
    xT = np.ascontiguousarray(xf.T.astype(bf16))           # [DIM, NTOK]

    fc = np.asarray(freqs_cis, np.float32)
    cosT = np.ascontiguousarray(fc[:, :, 0].T)             # [32, S]
    sinT = np.ascontiguousarray(fc[:, :, 1].T)
    cs4 = np.concatenate([cosT, sinT, cosT, sinT], axis=0)     # [128, S]
    cs4sw = np.concatenate([sinT, cosT, sinT, cosT], axis=0)

    wqf = np.asarray(wq, np.float32)                       # [H*QK, DIM]
    wkva = np.asarray(wkv_a, np.float32)                   # [C+R, DIM]
    wkvb = np.asarray(wkv_b, np.float32).reshape(H, NOPE + V, C)
    wof = np.asarray(wo, np.float32)                       # [DIM, H*V]
    wn = np.asarray(kv_norm_w, np.float32)                 # [C]

    kpe_blk = np.concatenate(
        [_deinterleave(wkva[C:C + R]), np.zeros((R, DIM), np.float32)], axis=0)

    ident = np.eye(P, dtype=bf16)
    ii = np.arange(P)
    tri = np.where(ii[:, None] <= ii[None, :], 0.0, NEG).astype(np.float32)
    maskA = np.zeros((P, HL, 2, P), np.float32)
    maskA[:, :, 0, :] = tri[:, None, :]
    maskB = np.full((P, HL, 2, P), NEG, np.float32)
    maskB[:, :, 1, :] = tri[:, None, :]
    maskA = maskA.astype(bf16)
    maskB = maskB.astype(bf16)

    in_maps = []
    for c in range(n_cores):
        h0 = c * HL
        wq_c = wqf.reshape(H, QK, DIM)[h0:h0 + HL] * scale  # [HL, QK, DIM]
        ropes = np.concatenate(
            [_deinterleave(wq_c[h, NOPE:]) for h in range(HL)], axis=0)
        blocks = [wkva[kc * P:(kc + 1) * P] for kc in range(CT)]
        blocks += [kpe_blk, ropes, wq_c[0, :NOPE], wq_c[1, :NOPE]]
        wall = np.stack(blocks, axis=0)                    # [NBLK, 128, DIM]
        wall = np.ascontiguousarray(
            wall.transpose(2, 0, 1)).astype(bf16)          # [DIM, NBLK, 128]

        b1 = (wkvb[h0:h0 + HL, :NOPE, :] * wn[None, None, :])  # [HL,128,C]
        wkvb1 = np.ascontiguousarray(b1.transpose(1, 0, 2)).astype(bf16)
        b2 = (wkvb[h0:h0 + HL, NOPE:, :] * wn[None, None, :])  # [HL,V,C]
        wkvb2 = np.ascontiguousarray(b2.transpose(2, 0, 1)).astype(bf16)
        woT_c = np.ascontiguousarray(
            wof[:, h0 * V:(h0 + HL) * V].T.reshape(HL, V, DIM)
            .transpose(1, 0, 2)).astype(bf16)              # [V, HL, DIM]
        in_maps.append({
            "xT": xT,
            "wall": wall,
            "wkvb1": wkvb1,
            "wkvb2": wkvb2,
            "woT": woT_c,
            "cs4": cs4,
            "cs4sw": cs4sw,
            "ident": ident,
            "maskA": maskA,
            "maskB": maskB,
        })
    return in_maps


_NC_CACHE = {}


def get_nc(s_per_b=S):
    if s_per_b not in _NC_CACHE:
        _NC_CACHE[s_per_b] = build_nc(s_per_b)
    return _NC_CACHE[s_per_b]


def kernel(x, freqs_cis, wq, wkv_a, wkv_b, wo, kv_norm_w, trace=False):
    nc = get_nc(S)
    in_maps = shard_inputs(x, freqs_cis, wq, wkv_a, wkv_b, wo, kv_norm_w)
    res = bass_utils.run_bass_kernel_spmd(
        nc, in_maps, core_ids=list(range(N_CORES)), trace=trace)
    y = res.results[0]["y"].astype(np.float64)
    for i in range(1, N_CORES):
        y += res.results[i]["y"]
    out = y.astype(np.float32).reshape(B, S, DIM)
    if trace:
        kernel.last_exec_time_ns = res.exec_time_ns
        kernel.last_results = res
    return out
